# revision 3
# baseline (speedup 1.0000x reference)
"""MLA self-attention block (eval mode) on 8 Trainium2 NeuronCores.

Sharding: tensor-parallel over heads (16 heads -> 2 per core), batch kept
whole per core.  The small d_latent KV projection is recomputed (replicated)
per core.  Each core computes a partial output through its two heads' slice
of w_o; the host sums the 8 partials.

Math (per core, heads h0=2c, h1=2c+1):
  xT            = x^T (PE transpose, per 128x128 tile)
  kvT  [L,T]    = w_dkv^T @ xT         (accumulated over C chunks)
  qT_h [S,T]    = w_q[:,h]^T @ xT
  q_latT [L,T]  = w_uk_h^T @ qT_h      (un-absorbed: (x@Wq)@Wuk == x@(Wq@Wuk))
  attT [s,q]    = kvT^T @ q_latT       (causal: only s <= q tiles computed)
  probs         = exp(scale*attT) * tri_mask   (logits are tiny -> no max-sub)
  yT  [S,q]     = v^T-ish: lhsT=v [s,S], rhs=probs [s,q], accumulate over s
  den [1,q]     = ones^T @ probs
  yn            = yT * (1/den broadcast)
  out_partial   = yn^T @ w_o_rows (2 heads accumulated)

All matmuls run in float32r (full PE rate at N>=256, ~1.5e-4 rel err).
"""

import sys
import os

sys.path.insert(0, "/opt/trn_rl_repo")

import numpy as np
from contextlib import ExitStack

import concourse.bass as bass
import concourse.tile as tile
from concourse import bacc, mybir
from concourse import bass_utils

F32 = mybir.dt.float32
F32R = mybir.dt.float32r

B, T, C = 2, 2048, 2048
H, S, L = 16, 128, 512
NCORES = 8
HPC = H // NCORES  # 2 heads per core
NT = T // 512  # 4 t-chunks of 512
SCALE = float(1.0 / np.sqrt(np.float32(C)))

_CACHE = {}


def _build():
    nc = bacc.Bacc("TRN2", target_bir_lowering=False, debug=False, num_devices=NCORES)

    x_ap = nc.dram_tensor("x", [B, T, C], F32, kind="ExternalInput").ap()
    w_dkv = nc.dram_tensor("w_dkv", [C, L], F32, kind="ExternalInput").ap()
    w_q_sl = nc.dram_tensor("w_q_sl", [C, HPC * S], F32, kind="ExternalInput").ap()
    w_uk_sl = nc.dram_tensor("w_uk_sl", [HPC * S, L], F32, kind="ExternalInput").ap()
    w_uv_sl = nc.dram_tensor("w_uv_sl", [L, HPC * S], F32, kind="ExternalInput").ap()
    w_o_sl = nc.dram_tensor("w_o_sl", [HPC * S, C], F32, kind="ExternalInput").ap()
    ident_d = nc.dram_tensor("ident", [128, 128], F32, kind="ExternalInput").ap()
    tri_d = nc.dram_tensor("tri", [128, 128], F32, kind="ExternalInput").ap()
    onesc_d = nc.dram_tensor("ones_col", [128, 1], F32, kind="ExternalInput").ap()
    onesr_d = nc.dram_tensor("ones_row", [1, 128], F32, kind="ExternalInput").ap()
    out_ap = nc.dram_tensor("out", [B, T, C], F32, kind="ExternalOutput").ap()

    with tile.TileContext(nc) as tc:
        with ExitStack() as ctx:
            wpool = ctx.enter_context(tc.tile_pool(name="w", bufs=1))
            pers = ctx.enter_context(tc.tile_pool(name="pers", bufs=1))
            sb2 = ctx.enter_context(tc.tile_pool(name="sb2", bufs=2))
            sb3 = ctx.enter_context(tc.tile_pool(name="sb3", bufs=3))
            sb4 = ctx.enter_context(tc.tile_pool(name="sb4", bufs=4))
            psA = ctx.enter_context(tc.tile_pool(name="psA", bufs=4, space="PSUM"))
            psB = ctx.enter_context(tc.tile_pool(name="psB", bufs=2, space="PSUM"))
            psC = ctx.enter_context(tc.tile_pool(name="psC", bufs=2, space="PSUM"))

            # ---- weights / constants (resident) ----
            wdkv = wpool.tile([128, 16, L], F32R, tag="wdkv")
            nc.sync.dma_start(
                wdkv[:], w_dkv.rearrange("(cc p) l -> p cc l", p=128).bitcast(F32R)
            )
            wq = wpool.tile([128, 16, HPC * S], F32R, tag="wq")
            nc.sync.dma_start(
                wq[:], w_q_sl.rearrange("(cc p) f -> p cc f", p=128).bitcast(F32R)
            )
            wuk = wpool.tile([128, HPC, L], F32R, tag="wuk")
            nc.sync.dma_start(
                wuk[:], w_uk_sl.rearrange("(h p) l -> p h l", p=128).bitcast(F32R)
            )
            wuv = wpool.tile([128, 4, HPC * S], F32R, tag="wuv")
            nc.sync.dma_start(
                wuv[:], w_uv_sl.rearrange("(lc p) f -> p lc f", p=128).bitcast(F32R)
            )
            wo = wpool.tile([128, HPC, C], F32R, tag="wo")
            nc.sync.dma_start(
                wo[:], w_o_sl.rearrange("(h p) f -> p h f", p=128).bitcast(F32R)
            )
            ident = wpool.tile([128, 128], F32R, tag="ident")
            nc.sync.dma_start(ident[:], ident_d.bitcast(F32R))
            tri = wpool.tile([128, 128], F32R, tag="tri")
            nc.sync.dma_start(tri[:], tri_d.bitcast(F32R))
            onesc = wpool.tile([128, 1], F32R, tag="onesc")
            nc.sync.dma_start(onesc[:], onesc_d.bitcast(F32R))
            onesr = wpool.tile([1, 128], F32R, tag="onesr")
            nc.sync.dma_start(onesr[:], onesr_d.bitcast(F32R))

            for b in range(B):
                kvT = pers.tile([128, 4, T], F32R, tag="kvT")
                vsb = pers.tile([128, T // 128, HPC * S], F32R, tag="vsb")

                for j in range(NT):
                    t0 = j * 512

                    # ======== phase 1: xT, kvT, qT for this t-chunk ========
                    kvps = [psA.tile([128, 512], F32, tag="acc4", name=f"kvps{_}") for _ in range(4)]
                    qps = [psB.tile([128, 512], F32, tag="acc2", name=f"qps{_}") for _ in range(HPC)]
                    xt_sb = [None] * 16

                    def do_transpose(cc):
                        xn = sb3.tile([128, 4, 128], F32R, tag="xnat")
                        nc.sync.dma_start(
                            xn[:],
                            x_ap[b, t0 : t0 + 512, cc * 128 : (cc + 1) * 128]
                            .rearrange("(tt p) c -> p tt c", p=128)
                            .bitcast(F32R),
                        )
                        xtp = psC.tile([128, 512], F32R, tag="misc2")
                        for tt in range(4):
                            nc.tensor.transpose(
                                xtp[:, tt * 128 : (tt + 1) * 128],
                                xn[:, tt, :],
                                ident[:],
                            )
                        xt = sb3.tile([128, 512], F32R, tag="xt")
                        nc.vector.tensor_copy(xt[:], xtp[:])
                        xt_sb[cc] = xt

                    def do_mms(cc):
                        xt = xt_sb[cc]
                        for lc in range(4):
                            nc.tensor.matmul(
                                kvps[lc][:],
                                wdkv[:, cc, lc * 128 : (lc + 1) * 128],
                                xt[:],
                                start=(cc == 0),
                                stop=(cc == 15),
                            )
                        for h in range(HPC):
                            nc.tensor.matmul(
                                qps[h][:],
                                wq[:, cc, h * S : (h + 1) * S],
                                xt[:],
                                start=(cc == 0),
                                stop=(cc == 15),
                            )

                    do_transpose(0)
                    for cc in range(1, 16):
                        do_transpose(cc)
                        do_mms(cc - 1)
                    do_mms(15)

                    for lc in range(4):
                        nc.vector.tensor_copy(kvT[:, lc, t0 : t0 + 512], kvps[lc][:])
                    qT = []
                    for h in range(HPC):
                        qt = sb2.tile([128, 512], F32R, tag="qT")
                        nc.scalar.activation(
                            qt[:], qps[h][:], mybir.ActivationFunctionType.Copy
                        )
                        qT.append(qt)

                    # q_latT for this chunk: [L, 512] per head
                    qlat = []
                    for h in range(HPC):
                        ql = sb3.tile([128, 4, 512], F32R, tag="qlat")
                        for lc in range(4):
                            qlp = psB.tile([128, 512], F32, tag="acc2")
                            nc.tensor.matmul(
                                qlp[:],
                                wuk[:, h, lc * 128 : (lc + 1) * 128],
                                qT[h][:],
                                start=True,
                                stop=True,
                            )
                            nc.vector.tensor_copy(ql[:, lc, :], qlp[:])
                        qlat.append(ql)

                    # v rows for this chunk (both heads together, N=256)
                    for tt in range(4):
                        vp = psB.tile([128, HPC * S], F32, tag="acc2")
                        for lc in range(4):
                            nc.tensor.matmul(
                                vp[:],
                                kvT[:, lc, t0 + tt * 128 : t0 + (tt + 1) * 128],
                                wuv[:, lc, :],
                                start=(lc == 0),
                                stop=(lc == 3),
                            )
                        nc.vector.tensor_copy(vsb[:, 4 * j + tt, :], vp[:])

                    # ======== attention for this q-chunk ========
                    yn = []
                    for h in range(HPC):
                        yps = psB.tile([128, 512], F32, tag="acc2")
                        dps = psC.tile([1, 512], F32, tag="misc2")
                        nst = 4 * j + 4
                        for i in range(nst):
                            n0 = (i - 4 * j) * 128 if i >= 4 * j else 0
                            aps = psA.tile([128, 512], F32, tag="acc4")
                            for lc in range(4):
                                nc.tensor.matmul(
                                    aps[:, n0:512],
                                    kvT[:, lc, i * 128 : (i + 1) * 128],
                                    qlat[h][:, lc, n0:512],
                                    start=(lc == 0),
                                    stop=(lc == 3),
                                )
                            ex = sb4.tile([128, 512], F32R, tag="exp")
                            nc.scalar.activation(
                                ex[:, n0:512],
                                aps[:, n0:512],
                                mybir.ActivationFunctionType.Exp,
                                scale=SCALE,
                            )
                            if i >= 4 * j:
                                nc.vector.tensor_mul(
                                    ex[:, n0 : n0 + 128],
                                    ex[:, n0 : n0 + 128],
                                    tri[:],
                                )
                            nc.tensor.matmul(
                                yps[:, n0:512],
                                vsb[:, i, h * S : (h + 1) * S],
                                ex[:, n0:512],
                                start=(i == 0),
                                stop=(i == nst - 1),
                            )
                            nc.tensor.matmul(
                                dps[:, n0:512],
                                onesc[:],
                                ex[:, n0:512],
                                start=(i == 0),
                                stop=(i == nst - 1),
                            )
                        rec = sb2.tile([1, 512], F32R, tag="rec")
                        with nc.allow_low_precision(reason="f32r is fp32-width"):
                            nc.vector.reciprocal(rec[:], dps[:])
                        bps = psA.tile([128, 512], F32, tag="acc4")
                        nc.tensor.matmul(bps[:], onesr[:], rec[:], start=True, stop=True)
                        bcs = sb2.tile([128, 512], F32, tag="bcs")
                        nc.scalar.activation(
                            bcs[:], bps[:], mybir.ActivationFunctionType.Copy
                        )
                        y = sb4.tile([128, 512], F32R, tag="yn")
                        with nc.allow_low_precision(reason="f32r is fp32-width"):
                            nc.vector.tensor_mul(y[:], yps[:], bcs[:])
                        yn.append(y)

                    # ======== output partial for this t-chunk ========
                    for tt in range(4):
                        for ncx in range(4):
                            op = psA.tile([128, 512], F32, tag="acc4")
                            for h in range(HPC):
                                nc.tensor.matmul(
                                    op[:],
                                    yn[h][:, tt * 128 : (tt + 1) * 128],
                                    wo[:, h, ncx * 512 : (ncx + 1) * 512],
                                    start=(h == 0),
                                    stop=(h == HPC - 1),
                                )
                            osb = sb4.tile([128, 512], F32, tag="osb")
                            nc.vector.tensor_copy(osb[:], op[:])
                            nc.sync.dma_start(
                                out_ap[
                                    b,
                                    t0 + tt * 128 : t0 + (tt + 1) * 128,
                                    ncx * 512 : (ncx + 1) * 512,
                                ],
                                osb[:],
                            )

    nc.compile()
    return nc


def _get_nc():
    if "nc" not in _CACHE:
        _CACHE["nc"] = _build()
    return _CACHE["nc"]


def kernel(x, w_dkv, w_uk, w_uv, w_q, w_o):
    x = np.ascontiguousarray(np.asarray(x, dtype=np.float32))
    w_dkv = np.ascontiguousarray(np.asarray(w_dkv, dtype=np.float32))
    w_uk = np.ascontiguousarray(np.asarray(w_uk, dtype=np.float32))
    w_uv = np.ascontiguousarray(np.asarray(w_uv, dtype=np.float32))
    w_q = np.ascontiguousarray(np.asarray(w_q, dtype=np.float32))
    w_o = np.ascontiguousarray(np.asarray(w_o, dtype=np.float32))

    nc = _get_nc()

    ident = np.eye(128, dtype=np.float32)
    tri = np.triu(np.ones((128, 128), dtype=np.float32))
    ones_col = np.ones((128, 1), dtype=np.float32)
    ones_row = np.ones((1, 128), dtype=np.float32)

    in_maps = []
    for c in range(NCORES):
        sl = slice(c * HPC * S, (c + 1) * HPC * S)
        in_maps.append(
            {
                "x": x,
                "w_dkv": w_dkv,
                "w_q_sl": np.ascontiguousarray(w_q[:, sl]),
                "w_uk_sl": np.ascontiguousarray(w_uk[sl, :]),
                "w_uv_sl": np.ascontiguousarray(w_uv[:, sl]),
                "w_o_sl": np.ascontiguousarray(w_o[sl, :]),
                "ident": ident,
                "tri": tri,
                "ones_col": ones_col,
                "ones_row": ones_row,
            }
        )

    kwargs = dict(_CACHE.get("run_kwargs", {}))
    res = bass_utils.run_bass_kernel_spmd(
        nc, in_maps, core_ids=list(range(NCORES)), **kwargs
    )
    _CACHE["last_result"] = res

    acc = np.zeros((B, T, C), dtype=np.float64)
    for r in res.results:
        acc += r["out"]
    return acc.astype(np.float32)


# revision 6
# speedup vs baseline: 1.0811x; 1.0811x over previous
"""MLA self-attention block (eval mode) on 8 Trainium2 NeuronCores.

Sharding: tensor-parallel over heads (16 heads -> 2 per core), batch kept
whole per core.  The small d_latent KV projection is recomputed (replicated)
per core.  Each core computes a partial output through its two heads' slice
of w_o; the host sums the 8 partials.

Math (per core, heads h0=2c, h1=2c+1):
  xT            = x^T (PE transpose, per 128x128 tile)
  kvT  [L,T]    = w_dkv^T @ xT         (accumulated over C chunks)
  qT_h [S,T]    = w_q[:,h]^T @ xT
  q_latT [L,T]  = w_uk_h^T @ qT_h      (un-absorbed: (x@Wq)@Wuk == x@(Wq@Wuk))
  attT [s,q]    = kvT^T @ q_latT       (causal: only s <= q tiles computed)
  probs         = exp(scale*attT) * tri_mask   (logits are tiny -> no max-sub)
  yT  [S,q]     = lhsT=v [s,S], rhs=probs [s,q], accumulated over s
  den [1,q]     = ones^T @ probs
  yn            = yT * (1/den broadcast via ones-outer-product matmul)
  out_partial   = yn^T @ w_o_rows (2 heads accumulated)

All matmuls run in float32r (full PE rate at N>=256, ~1.5e-4 rel err).
Output DMA goes through the (otherwise idle) GpSimd SWDGE queue so x-tile
prefetches on the Sync HWDGE queue are never stuck behind stores.
"""

import sys
import os

sys.path.insert(0, "/opt/trn_rl_repo")

import numpy as np
from contextlib import ExitStack

import concourse.bass as bass
import concourse.tile as tile
from concourse import bacc, mybir
from concourse import bass_utils

F32 = mybir.dt.float32
F32R = mybir.dt.float32r

B, T, C = 2, 2048, 2048
H, S, L = 16, 128, 512
NCORES = 8
HPC = H // NCORES  # 2 heads per core
NT = T // 512  # 4 t-chunks of 512
SCALE = float(1.0 / np.sqrt(np.float32(C)))

_CACHE = {}


def _build():
    nc = bacc.Bacc("TRN2", target_bir_lowering=False, debug=False, num_devices=NCORES)

    x_ap = nc.dram_tensor("x", [B, T, C], F32, kind="ExternalInput").ap()
    w_dkv = nc.dram_tensor("w_dkv", [C, L], F32, kind="ExternalInput").ap()
    w_q_sl = nc.dram_tensor("w_q_sl", [C, HPC * S], F32, kind="ExternalInput").ap()
    w_uk_sl = nc.dram_tensor("w_uk_sl", [HPC * S, L], F32, kind="ExternalInput").ap()
    w_uv_sl = nc.dram_tensor("w_uv_sl", [L, HPC * S], F32, kind="ExternalInput").ap()
    w_o_sl = nc.dram_tensor("w_o_sl", [HPC * S, C], F32, kind="ExternalInput").ap()
    ident_d = nc.dram_tensor("ident", [128, 128], F32, kind="ExternalInput").ap()
    tri_d = nc.dram_tensor("tri", [128, 128], F32, kind="ExternalInput").ap()
    onesc_d = nc.dram_tensor("ones_col", [128, 1], F32, kind="ExternalInput").ap()
    onesr_d = nc.dram_tensor("ones_row", [1, 128], F32, kind="ExternalInput").ap()
    out_ap = nc.dram_tensor("out", [B, T, C], F32, kind="ExternalOutput").ap()

    w_dkv_r = w_dkv.rearrange("(cc p) l -> p cc l", p=128).bitcast(F32R)
    w_q_r = w_q_sl.rearrange("(cc p) f -> p cc f", p=128).bitcast(F32R)

    with tile.TileContext(nc) as tc:
        with ExitStack() as ctx:
            wpool = ctx.enter_context(tc.tile_pool(name="w", bufs=1))
            pers = ctx.enter_context(tc.tile_pool(name="pers", bufs=1))
            sb2 = ctx.enter_context(tc.tile_pool(name="sb2", bufs=2))
            sb4 = ctx.enter_context(tc.tile_pool(name="sb4", bufs=4))
            psA = ctx.enter_context(tc.tile_pool(name="psA", bufs=4, space="PSUM"))
            psB = ctx.enter_context(tc.tile_pool(name="psB", bufs=2, space="PSUM"))
            psC = ctx.enter_context(tc.tile_pool(name="psC", bufs=2, space="PSUM"))

            # ---- constants + phase-1 weights on the fast Sync queue,
            #      per-c-chunk so the first matmuls start after ~0.5 MB ----
            ident = wpool.tile([128, 128], F32R, tag="ident", name="ident")
            nc.sync.dma_start(ident[:], ident_d.bitcast(F32R))
            wdkv_t = []
            wq_t = []
            for cc in range(16):
                wd = wpool.tile([128, L], F32R, tag=f"wdkv{cc}", name=f"wdkv{cc}")
                nc.sync.dma_start(wd[:], w_dkv_r[:, cc, :])
                wdkv_t.append(wd)
                wqc = wpool.tile([128, HPC * S], F32R, tag=f"wq{cc}", name=f"wq{cc}")
                nc.sync.dma_start(wqc[:], w_q_r[:, cc, :])
                wq_t.append(wqc)

            # ---- later-phase weights on the GpSimd (SWDGE) queue ----
            wuk = wpool.tile([128, HPC, L], F32R, tag="wuk", name="wuk")
            nc.gpsimd.dma_start(
                wuk[:], w_uk_sl.rearrange("(h p) l -> p h l", p=128).bitcast(F32R)
            )
            wuv = wpool.tile([128, 4, HPC * S], F32R, tag="wuv", name="wuv")
            nc.gpsimd.dma_start(
                wuv[:], w_uv_sl.rearrange("(lc p) f -> p lc f", p=128).bitcast(F32R)
            )
            wo = wpool.tile([128, HPC, C], F32R, tag="wo", name="wo")
            nc.gpsimd.dma_start(
                wo[:], w_o_sl.rearrange("(h p) f -> p h f", p=128).bitcast(F32R)
            )
            tri = wpool.tile([128, 128], F32R, tag="tri", name="tri")
            nc.gpsimd.dma_start(tri[:], tri_d.bitcast(F32R))
            onesc = wpool.tile([128, 1], F32R, tag="onesc", name="onesc")
            nc.gpsimd.dma_start(onesc[:], onesc_d.bitcast(F32R))
            onesr = wpool.tile([1, 128], F32R, tag="onesr", name="onesr")
            nc.gpsimd.dma_start(onesr[:], onesr_d.bitcast(F32R))

            pending_out = []  # deferred output-projection work items

            def emit_out(item):
                bb, jj, yn_ = item
                tb = jj * 512
                for tt in range(4):
                    for ncx in range(4):
                        op = psA.tile([128, 512], F32, tag="acc4", name="op")
                        for h in range(HPC):
                            nc.tensor.matmul(
                                op[:],
                                yn_[h][:, tt * 128 : (tt + 1) * 128],
                                wo[:, h, ncx * 512 : (ncx + 1) * 512],
                                start=(h == 0),
                                stop=(h == HPC - 1),
                            )
                        osb = sb4.tile([128, 512], F32, tag="osb", name="osb")
                        nc.vector.tensor_copy(osb[:], op[:])
                        nc.gpsimd.dma_start(
                            out_ap[
                                bb,
                                tb + tt * 128 : tb + (tt + 1) * 128,
                                ncx * 512 : (ncx + 1) * 512,
                            ],
                            osb[:],
                        )

            for b in range(B):
                kvT = pers.tile([128, 4, T], F32R, tag="kvT", name="kvT")
                vsb = pers.tile([128, T // 128, HPC * S], F32R, tag="vsb", name="vsb")

                for j in range(NT):
                    t0 = j * 512

                    # ======== phase 1: xT, kvT, qT for this t-chunk ========
                    kvps = []
                    qps = []
                    xt_sb = [None] * 16

                    def do_transpose(cc):
                        xn = sb4.tile([128, 4, 128], F32R, tag="xnat", name="xn")
                        nc.sync.dma_start(
                            xn[:],
                            x_ap[b, t0 : t0 + 512, cc * 128 : (cc + 1) * 128]
                            .rearrange("(tt p) c -> p tt c", p=128)
                            .bitcast(F32R),
                        )
                        xtp = psC.tile([128, 512], F32R, tag="misc2", name="xtp")
                        for tt in range(4):
                            nc.tensor.transpose(
                                xtp[:, tt * 128 : (tt + 1) * 128],
                                xn[:, tt, :],
                                ident[:],
                            )
                        xt = sb4.tile([128, 512], F32R, tag="xt", name="xt")
                        nc.vector.tensor_copy(xt[:], xtp[:])
                        xt_sb[cc] = xt

                    def do_mms(cc):
                        xt = xt_sb[cc]
                        for lc in range(4):
                            nc.tensor.matmul(
                                kvps[lc][:],
                                wdkv_t[cc][:, lc * 128 : (lc + 1) * 128],
                                xt[:],
                                start=(cc == 0),
                                stop=(cc == 15),
                            )
                        for h in range(HPC):
                            nc.tensor.matmul(
                                qps[h][:],
                                wq_t[cc][:, h * S : (h + 1) * S],
                                xt[:],
                                start=(cc == 0),
                                stop=(cc == 15),
                            )

                    # transpose 3 chunks ahead; slot the deferred output
                    # projection of the previous t-chunk into the bubble
                    do_transpose(0)
                    do_transpose(1)
                    do_transpose(2)
                    if pending_out:
                        emit_out(pending_out.pop())
                    kvps.extend(
                        psA.tile([128, 512], F32, tag="acc4", name=f"kvps{i}")
                        for i in range(4)
                    )
                    qps.extend(
                        psB.tile([128, 512], F32, tag="acc2", name=f"qps{i}")
                        for i in range(HPC)
                    )
                    for cc in range(3, 16):
                        do_transpose(cc)
                        do_mms(cc - 3)
                    for cc in range(13, 16):
                        do_mms(cc)

                    for lc in range(4):
                        nc.vector.tensor_copy(kvT[:, lc, t0 : t0 + 512], kvps[lc][:])
                    qT = []
                    for h in range(HPC):
                        qt = sb2.tile([128, 512], F32R, tag="qT", name="qt")
                        nc.scalar.activation(
                            qt[:], qps[h][:], mybir.ActivationFunctionType.Copy
                        )
                        qT.append(qt)

                    # q_latT for this chunk: [L, 512] per head
                    qlat = []
                    for h in range(HPC):
                        ql = sb2.tile([128, 4, 512], F32R, tag="qlat", name="ql")
                        for lc in range(4):
                            qlp = psB.tile([128, 512], F32, tag="acc2", name="qlp")
                            nc.tensor.matmul(
                                qlp[:],
                                wuk[:, h, lc * 128 : (lc + 1) * 128],
                                qT[h][:],
                                start=True,
                                stop=True,
                            )
                            nc.vector.tensor_copy(ql[:, lc, :], qlp[:])
                        qlat.append(ql)

                    # v rows for this chunk (both heads together, N=256)
                    for tt in range(4):
                        vp = psB.tile([128, HPC * S], F32, tag="acc2", name="vp")
                        for lc in range(4):
                            nc.tensor.matmul(
                                vp[:],
                                kvT[:, lc, t0 + tt * 128 : t0 + (tt + 1) * 128],
                                wuv[:, lc, :],
                                start=(lc == 0),
                                stop=(lc == 3),
                            )
                        nc.vector.tensor_copy(vsb[:, 4 * j + tt, :], vp[:])

                    # ======== attention for this q-chunk ========
                    yn = []
                    for h in range(HPC):
                        yps = psB.tile([128, 512], F32, tag="acc2", name="yps")
                        dps = psC.tile([1, 512], F32, tag="misc2", name="dps")
                        nst = 4 * j + 4

                        def y_den(item):
                            i, n0, ex = item
                            nc.tensor.matmul(
                                yps[:, n0:512],
                                vsb[:, i, h * S : (h + 1) * S],
                                ex[:, n0:512],
                                start=(i == 0),
                                stop=(i == nst - 1),
                            )
                            nc.tensor.matmul(
                                dps[:, n0:512],
                                onesc[:],
                                ex[:, n0:512],
                                start=(i == 0),
                                stop=(i == nst - 1),
                            )

                        prev = None
                        for i in range(nst):
                            n0 = (i - 4 * j) * 128 if i >= 4 * j else 0
                            aps = psA.tile([128, 512], F32, tag="acc4", name="aps")
                            for lc in range(4):
                                nc.tensor.matmul(
                                    aps[:, n0:512],
                                    kvT[:, lc, i * 128 : (i + 1) * 128],
                                    qlat[h][:, lc, n0:512],
                                    start=(lc == 0),
                                    stop=(lc == 3),
                                )
                            ex = sb4.tile([128, 512], F32R, tag="exp", name="ex")
                            nc.scalar.activation(
                                ex[:, n0:512],
                                aps[:, n0:512],
                                mybir.ActivationFunctionType.Exp,
                                scale=SCALE,
                            )
                            if i >= 4 * j:
                                nc.vector.tensor_mul(
                                    ex[:, n0 : n0 + 128],
                                    ex[:, n0 : n0 + 128],
                                    tri[:],
                                )
                            if prev is not None:
                                y_den(prev)
                            prev = (i, n0, ex)
                        y_den(prev)

                        rec32 = sb2.tile([1, 512], F32, tag="rec32", name="rec32")
                        nc.vector.reciprocal_approx_fast(rec32[:], dps[:])
                        rec = sb2.tile([1, 512], F32R, tag="rec", name="rec")
                        nc.vector.tensor_copy(rec[:], rec32[:])
                        bps = psA.tile([128, 512], F32, tag="acc4", name="bps")
                        nc.tensor.matmul(bps[:], onesr[:], rec[:], start=True, stop=True)
                        bcs = sb2.tile([128, 512], F32, tag="bcs", name="bcs")
                        nc.scalar.activation(
                            bcs[:], bps[:], mybir.ActivationFunctionType.Copy
                        )
                        y = sb4.tile([128, 512], F32R, tag="yn", name="y")
                        with nc.allow_low_precision(reason="f32r is fp32-width"):
                            nc.vector.tensor_mul(y[:], yps[:], bcs[:])
                        yn.append(y)

                    pending_out.append((b, j, yn))

            emit_out(pending_out.pop())

    nc.compile()
    return nc


def _get_nc():
    if "nc" not in _CACHE:
        _CACHE["nc"] = _build()
    return _CACHE["nc"]


def kernel(x, w_dkv, w_uk, w_uv, w_q, w_o):
    x = np.ascontiguousarray(np.asarray(x, dtype=np.float32))
    w_dkv = np.ascontiguousarray(np.asarray(w_dkv, dtype=np.float32))
    w_uk = np.ascontiguousarray(np.asarray(w_uk, dtype=np.float32))
    w_uv = np.ascontiguousarray(np.asarray(w_uv, dtype=np.float32))
    w_q = np.ascontiguousarray(np.asarray(w_q, dtype=np.float32))
    w_o = np.ascontiguousarray(np.asarray(w_o, dtype=np.float32))

    nc = _get_nc()

    ident = np.eye(128, dtype=np.float32)
    tri = np.triu(np.ones((128, 128), dtype=np.float32))
    ones_col = np.ones((128, 1), dtype=np.float32)
    ones_row = np.ones((1, 128), dtype=np.float32)

    in_maps = []
    for c in range(NCORES):
        sl = slice(c * HPC * S, (c + 1) * HPC * S)
        in_maps.append(
            {
                "x": x,
                "w_dkv": w_dkv,
                "w_q_sl": np.ascontiguousarray(w_q[:, sl]),
                "w_uk_sl": np.ascontiguousarray(w_uk[sl, :]),
                "w_uv_sl": np.ascontiguousarray(w_uv[:, sl]),
                "w_o_sl": np.ascontiguousarray(w_o[sl, :]),
                "ident": ident,
                "tri": tri,
                "ones_col": ones_col,
                "ones_row": ones_row,
            }
        )

    kwargs = dict(_CACHE.get("run_kwargs", {}))
    res = bass_utils.run_bass_kernel_spmd(
        nc, in_maps, core_ids=list(range(NCORES)), **kwargs
    )
    _CACHE["last_result"] = res

    acc = np.zeros((B, T, C), dtype=np.float64)
    for r in res.results:
        acc += r["out"]
    return acc.astype(np.float32)


# revision 7
# speedup vs baseline: 1.1312x; 1.0463x over previous
"""MLA self-attention block (eval mode) on 8 Trainium2 NeuronCores.

Sharding: tensor-parallel over heads (16 heads -> 2 per core), batch kept
whole per core.  The small d_latent KV projection is recomputed (replicated)
per core.  Each core computes a partial output through its two heads' slice
of w_o; the host sums the 8 partials.

Math (per core, heads h0=2c, h1=2c+1):
  xT            = x^T (PE transpose, per 128x128 tile)
  kvT  [L,T]    = w_dkv^T @ xT         (accumulated over C chunks)
  qT_h [S,T]    = w_q[:,h]^T @ xT
  q_latT [L,T]  = w_uk_h^T @ qT_h      (un-absorbed: (x@Wq)@Wuk == x@(Wq@Wuk))
  attT [s,q]    = kvT^T @ q_latT       (causal: only s <= q tiles computed)
  probs         = exp(scale*attT) * tri_mask   (logits are tiny -> no max-sub)
  yT  [S,q]     = lhsT=v [s,S], rhs=probs [s,q], accumulated over s
  den [1,q]     = ones^T @ probs
  yn            = yT * (1/den broadcast via ones-outer-product matmul)
  out_partial   = yn^T @ w_o_rows (2 heads accumulated)

All matmuls run in float32r (full PE rate at N>=256, ~1.5e-4 rel err).
Output DMA goes through the (otherwise idle) GpSimd SWDGE queue so x-tile
prefetches on the Sync HWDGE queue are never stuck behind stores.
"""

import sys
import os

sys.path.insert(0, "/opt/trn_rl_repo")

import numpy as np
from contextlib import ExitStack

import concourse.bass as bass
import concourse.tile as tile
from concourse import bacc, mybir
from concourse import bass_utils

F32 = mybir.dt.float32
F32R = mybir.dt.float32r

B, T, C = 2, 2048, 2048
H, S, L = 16, 128, 512
NCORES = 8
HPC = H // NCORES  # 2 heads per core
NT = T // 512  # 4 t-chunks of 512
SCALE = float(1.0 / np.sqrt(np.float32(C)))

_CACHE = {}


def _build():
    nc = bacc.Bacc("TRN2", target_bir_lowering=False, debug=False, num_devices=NCORES)

    x_ap = nc.dram_tensor("x", [B, T, C], F32, kind="ExternalInput").ap()
    w_dkv = nc.dram_tensor("w_dkv", [C, L], F32, kind="ExternalInput").ap()
    w_q_sl = nc.dram_tensor("w_q_sl", [C, HPC * S], F32, kind="ExternalInput").ap()
    w_uk_sl = nc.dram_tensor("w_uk_sl", [HPC * S, L], F32, kind="ExternalInput").ap()
    w_uv_sl = nc.dram_tensor("w_uv_sl", [L, HPC * S], F32, kind="ExternalInput").ap()
    w_o_sl = nc.dram_tensor("w_o_sl", [HPC * S, C], F32, kind="ExternalInput").ap()
    ident_d = nc.dram_tensor("ident", [128, 128], F32, kind="ExternalInput").ap()
    tri_d = nc.dram_tensor("tri", [128, 128], F32, kind="ExternalInput").ap()
    onesc_d = nc.dram_tensor("ones_col", [128, 1], F32, kind="ExternalInput").ap()
    onesr_d = nc.dram_tensor("ones_row", [1, 128], F32, kind="ExternalInput").ap()
    out_ap = nc.dram_tensor("out", [B, T, C], F32, kind="ExternalOutput").ap()

    w_dkv_r = w_dkv.rearrange("(cc p) l -> p cc l", p=128).bitcast(F32R)
    w_q_r = w_q_sl.rearrange("(cc p) f -> p cc f", p=128).bitcast(F32R)

    with tile.TileContext(nc) as tc:
        with ExitStack() as ctx:
            wpool = ctx.enter_context(tc.tile_pool(name="w", bufs=1))
            pers = ctx.enter_context(tc.tile_pool(name="pers", bufs=1))
            sb2 = ctx.enter_context(tc.tile_pool(name="sb2", bufs=2))
            sb4 = ctx.enter_context(tc.tile_pool(name="sb4", bufs=4))
            psA = ctx.enter_context(tc.tile_pool(name="psA", bufs=4, space="PSUM"))
            psB = ctx.enter_context(tc.tile_pool(name="psB", bufs=2, space="PSUM"))
            psC = ctx.enter_context(tc.tile_pool(name="psC", bufs=2, space="PSUM"))

            # ---- constants + phase-1 weights on the fast Sync queue,
            #      per-c-chunk so the first matmuls start after ~0.5 MB ----
            ident = wpool.tile([128, 128], F32R, tag="ident", name="ident")
            nc.scalar.dma_start(ident[:], ident_d.bitcast(F32R))
            wdkv_t = []
            wq_t = []
            for cc in range(16):
                wd = wpool.tile([128, L], F32R, tag=f"wdkv{cc}", name=f"wdkv{cc}")
                nc.scalar.dma_start(wd[:], w_dkv_r[:, cc, :])
                wdkv_t.append(wd)
                wqc = wpool.tile([128, HPC * S], F32R, tag=f"wq{cc}", name=f"wq{cc}")
                nc.scalar.dma_start(wqc[:], w_q_r[:, cc, :])
                wq_t.append(wqc)

            # ---- later-phase weights on the GpSimd (SWDGE) queue ----
            wuk = wpool.tile([128, HPC, L], F32R, tag="wuk", name="wuk")
            nc.gpsimd.dma_start(
                wuk[:], w_uk_sl.rearrange("(h p) l -> p h l", p=128).bitcast(F32R)
            )
            wuv = wpool.tile([128, 4, HPC * S], F32R, tag="wuv", name="wuv")
            nc.gpsimd.dma_start(
                wuv[:], w_uv_sl.rearrange("(lc p) f -> p lc f", p=128).bitcast(F32R)
            )
            wo = wpool.tile([128, HPC, C], F32R, tag="wo", name="wo")
            nc.gpsimd.dma_start(
                wo[:], w_o_sl.rearrange("(h p) f -> p h f", p=128).bitcast(F32R)
            )
            tri = wpool.tile([128, 128], F32R, tag="tri", name="tri")
            nc.gpsimd.dma_start(tri[:], tri_d.bitcast(F32R))
            onesc = wpool.tile([128, 1], F32R, tag="onesc", name="onesc")
            nc.gpsimd.dma_start(onesc[:], onesc_d.bitcast(F32R))
            onesr = wpool.tile([1, 128], F32R, tag="onesr", name="onesr")
            nc.gpsimd.dma_start(onesr[:], onesr_d.bitcast(F32R))

            pending_out = []  # deferred output-projection work items

            def emit_out(item):
                bb, jj, yn_ = item
                tb = jj * 512
                for tt in range(4):
                    for ncx in range(4):
                        op = psA.tile([128, 512], F32, tag="acc4", name="op")
                        for h in range(HPC):
                            nc.tensor.matmul(
                                op[:],
                                yn_[h][:, tt * 128 : (tt + 1) * 128],
                                wo[:, h, ncx * 512 : (ncx + 1) * 512],
                                start=(h == 0),
                                stop=(h == HPC - 1),
                            )
                        osb = sb4.tile([128, 512], F32, tag="osb", name="osb")
                        nc.vector.tensor_copy(osb[:], op[:])
                        nc.gpsimd.dma_start(
                            out_ap[
                                bb,
                                tb + tt * 128 : tb + (tt + 1) * 128,
                                ncx * 512 : (ncx + 1) * 512,
                            ],
                            osb[:],
                        )

            for b in range(B):
                kvT = pers.tile([128, 4, T], F32R, tag="kvT", name="kvT")
                vsb = pers.tile([128, T // 128, HPC * S], F32R, tag="vsb", name="vsb")

                for j in range(NT):
                    t0 = j * 512

                    # ======== phase 1: xT, kvT, qT for this t-chunk ========
                    kvps = []
                    qps = []
                    xt_sb = [None] * 16

                    def do_transpose(cc):
                        xn = sb4.tile([128, 4, 128], F32R, tag="xnat", name="xn")
                        nc.sync.dma_start(
                            xn[:],
                            x_ap[b, t0 : t0 + 512, cc * 128 : (cc + 1) * 128]
                            .rearrange("(tt p) c -> p tt c", p=128)
                            .bitcast(F32R),
                        )
                        xtp = psC.tile([128, 512], F32R, tag="misc2", name="xtp")
                        for tt in range(4):
                            nc.tensor.transpose(
                                xtp[:, tt * 128 : (tt + 1) * 128],
                                xn[:, tt, :],
                                ident[:],
                            )
                        xt = sb4.tile([128, 512], F32R, tag="xt", name="xt")
                        nc.vector.tensor_copy(xt[:], xtp[:])
                        xt_sb[cc] = xt

                    def do_mms(cc):
                        xt = xt_sb[cc]
                        for lc in range(4):
                            nc.tensor.matmul(
                                kvps[lc][:],
                                wdkv_t[cc][:, lc * 128 : (lc + 1) * 128],
                                xt[:],
                                start=(cc == 0),
                                stop=(cc == 15),
                            )
                        for h in range(HPC):
                            nc.tensor.matmul(
                                qps[h][:],
                                wq_t[cc][:, h * S : (h + 1) * S],
                                xt[:],
                                start=(cc == 0),
                                stop=(cc == 15),
                            )

                    # transpose 3 chunks ahead; slot the deferred output
                    # projection of the previous t-chunk into the bubble
                    do_transpose(0)
                    do_transpose(1)
                    do_transpose(2)
                    if pending_out:
                        emit_out(pending_out.pop())
                    kvps.extend(
                        psA.tile([128, 512], F32, tag="acc4", name=f"kvps{i}")
                        for i in range(4)
                    )
                    qps.extend(
                        psB.tile([128, 512], F32, tag="acc2", name=f"qps{i}")
                        for i in range(HPC)
                    )
                    for cc in range(3, 16):
                        do_transpose(cc)
                        do_mms(cc - 3)
                    for cc in range(13, 16):
                        do_mms(cc)

                    for lc in range(4):
                        nc.vector.tensor_copy(kvT[:, lc, t0 : t0 + 512], kvps[lc][:])
                    qT = []
                    for h in range(HPC):
                        qt = sb2.tile([128, 512], F32R, tag="qT", name="qt")
                        nc.scalar.activation(
                            qt[:], qps[h][:], mybir.ActivationFunctionType.Copy
                        )
                        qT.append(qt)

                    # q_latT for this chunk: [L, 512] per head
                    qlat = []
                    for h in range(HPC):
                        ql = sb2.tile([128, 4, 512], F32R, tag="qlat", name="ql")
                        for lc in range(4):
                            qlp = psB.tile([128, 512], F32, tag="acc2", name="qlp")
                            nc.tensor.matmul(
                                qlp[:],
                                wuk[:, h, lc * 128 : (lc + 1) * 128],
                                qT[h][:],
                                start=True,
                                stop=True,
                            )
                            nc.vector.tensor_copy(ql[:, lc, :], qlp[:])
                        qlat.append(ql)

                    # v rows for this chunk (both heads together, N=256)
                    for tt in range(4):
                        vp = psB.tile([128, HPC * S], F32, tag="acc2", name="vp")
                        for lc in range(4):
                            nc.tensor.matmul(
                                vp[:],
                                kvT[:, lc, t0 + tt * 128 : t0 + (tt + 1) * 128],
                                wuv[:, lc, :],
                                start=(lc == 0),
                                stop=(lc == 3),
                            )
                        nc.vector.tensor_copy(vsb[:, 4 * j + tt, :], vp[:])

                    # ======== attention for this q-chunk ========
                    yn = []
                    for h in range(HPC):
                        yps = psB.tile([128, 512], F32, tag="acc2", name="yps")
                        dps = psB.tile([1, 512], F32, tag="acc2", name="dps")
                        nst = 4 * j + 4

                        def y_den(item):
                            i, n0, ex = item
                            nc.tensor.matmul(
                                yps[:, n0:512],
                                vsb[:, i, h * S : (h + 1) * S],
                                ex[:, n0:512],
                                start=(i == 0),
                                stop=(i == nst - 1),
                            )
                            nc.tensor.matmul(
                                dps[:, n0:512],
                                onesc[:],
                                ex[:, n0:512],
                                start=(i == 0),
                                stop=(i == nst - 1),
                            )

                        prev = None
                        for i in range(nst):
                            n0 = (i - 4 * j) * 128 if i >= 4 * j else 0
                            aps = psA.tile([128, 512], F32, tag="acc4", name="aps")
                            for lc in range(4):
                                nc.tensor.matmul(
                                    aps[:, n0:512],
                                    kvT[:, lc, i * 128 : (i + 1) * 128],
                                    qlat[h][:, lc, n0:512],
                                    start=(lc == 0),
                                    stop=(lc == 3),
                                )
                            ex = sb4.tile([128, 512], F32R, tag="exp", name="ex")
                            nc.scalar.activation(
                                ex[:, n0:512],
                                aps[:, n0:512],
                                mybir.ActivationFunctionType.Exp,
                                scale=SCALE,
                            )
                            if i >= 4 * j:
                                nc.vector.tensor_mul(
                                    ex[:, n0 : n0 + 128],
                                    ex[:, n0 : n0 + 128],
                                    tri[:],
                                )
                            if prev is not None:
                                y_den(prev)
                            prev = (i, n0, ex)
                        y_den(prev)

                        rec32 = sb2.tile([1, 512], F32, tag="rec32", name="rec32")
                        nc.vector.reciprocal_approx_fast(rec32[:], dps[:])
                        rec = sb2.tile([1, 512], F32R, tag="rec", name="rec")
                        nc.vector.tensor_copy(rec[:], rec32[:])
                        bps = psA.tile([128, 512], F32, tag="acc4", name="bps")
                        nc.tensor.matmul(bps[:], onesr[:], rec[:], start=True, stop=True)
                        bcs = sb2.tile([128, 512], F32, tag="bcs", name="bcs")
                        nc.scalar.activation(
                            bcs[:], bps[:], mybir.ActivationFunctionType.Copy
                        )
                        y = sb4.tile([128, 512], F32R, tag="yn", name="y")
                        with nc.allow_low_precision(reason="f32r is fp32-width"):
                            nc.vector.tensor_mul(y[:], yps[:], bcs[:])
                        yn.append(y)

                    pending_out.append((b, j, yn))

            emit_out(pending_out.pop())

    nc.compile()
    return nc


def _get_nc():
    if "nc" not in _CACHE:
        _CACHE["nc"] = _build()
    return _CACHE["nc"]


def kernel(x, w_dkv, w_uk, w_uv, w_q, w_o):
    x = np.ascontiguousarray(np.asarray(x, dtype=np.float32))
    w_dkv = np.ascontiguousarray(np.asarray(w_dkv, dtype=np.float32))
    w_uk = np.ascontiguousarray(np.asarray(w_uk, dtype=np.float32))
    w_uv = np.ascontiguousarray(np.asarray(w_uv, dtype=np.float32))
    w_q = np.ascontiguousarray(np.asarray(w_q, dtype=np.float32))
    w_o = np.ascontiguousarray(np.asarray(w_o, dtype=np.float32))

    nc = _get_nc()

    ident = np.eye(128, dtype=np.float32)
    tri = np.triu(np.ones((128, 128), dtype=np.float32))
    ones_col = np.ones((128, 1), dtype=np.float32)
    ones_row = np.ones((1, 128), dtype=np.float32)

    in_maps = []
    for c in range(NCORES):
        sl = slice(c * HPC * S, (c + 1) * HPC * S)
        in_maps.append(
            {
                "x": x,
                "w_dkv": w_dkv,
                "w_q_sl": np.ascontiguousarray(w_q[:, sl]),
                "w_uk_sl": np.ascontiguousarray(w_uk[sl, :]),
                "w_uv_sl": np.ascontiguousarray(w_uv[:, sl]),
                "w_o_sl": np.ascontiguousarray(w_o[sl, :]),
                "ident": ident,
                "tri": tri,
                "ones_col": ones_col,
                "ones_row": ones_row,
            }
        )

    kwargs = dict(_CACHE.get("run_kwargs", {}))
    res = bass_utils.run_bass_kernel_spmd(
        nc, in_maps, core_ids=list(range(NCORES)), **kwargs
    )
    _CACHE["last_result"] = res

    acc = np.zeros((B, T, C), dtype=np.float64)
    for r in res.results:
        acc += r["out"]
    return acc.astype(np.float32)
